# revision 4
# baseline (speedup 1.0000x reference)
"""Hyperbolic MLR logits (Ganea et al.) on 8 Trainium2 NeuronCores.

Shapes (hardcoded): inp [128, 512] f32, p [2048, 512] f32, a [2048, 512] f32,
output [128, 2048] f32.

Math (same collapse as the original baseline): with c=1 the reference
    logit[b,k] = lam_p[k] * ||a_k|| * asinh(2<w,a_k>/(||a_k||(1-||w||^2))),
    w = (-p_k) (+)_mobius x_b
reduces via the gyrovector identity to
    logit[b,k] = (z^2 + PG0/PG1) * (z * lam[k]*PG1),
    z[b,k]     = vws[b]*qscale[k] + <W_k, xs_b>          (deg-3 asinh poly)
with all coefficient vectors host-precomputed (see _host_prep).

Design — driven by knockout probes on the real device (not the cost model):
  * The measured marginal body tracks total DMA BYTES and the NUMBER of
    input DMAs.  A second input DMA costs ~400-500ns at depth no matter
    which ring issues it, so everything rides in ONE bulk DMA per body.
  * Both matmul operands are fp8 e3m4 scaled x16 into its normal range
    (PSUM holds 256*z; the unscale is folded into host constants — all
    powers of two, so exact): wx bytes per core drop 384KB -> 288KB... per
    body total (wx 295KB + out 64KB) = 359KB vs the bf16 baseline's 451KB.
    End-to-end absmax-rel error: 1.715e-2 (HW bit-identical to the host
    simulation) vs the 2e-2 gate.
  * The rank-1 z-term and the lam broadcast ride as bf16 "rider rows"
    inside the wx DMA (a 768B column strip; only partitions 0/64/65 carry
    data).  Rider matmuls need lhsT/rhs at the SAME base partition (in
    {0,32,64}), hence the slot-A (lhsT) / slot-BC (rhs) layout.
  * The asinh chain is balanced ACT/DVE, each ~2 ops/body:
      ACT zc = copy(mm)  (256*z), ACT u = square(mm)  (65536*z^2),
      DVE zl = zc * lam_ps  (lam rides as lam*PG1/2^24),
      DVE o  = (u + (PG0/PG1)*65536) * zl  -> bf16.
    Each instruction needs at most one fresh semaphore (in-order rings
    cover the rest).
  * Two PSUM banks per body (mm + lam_ps), bufs=4 -> 4-deep rotation;
    SBUF pool 6-deep.  PE order: z-rider, lam riders, then the 4 mains
    (mixed dtype is allowed; the mains are e3m4 x e3m4).
  * Output via the Pool/SWDGE ring (the two HWDGE rings would serialize).

Measured (n-sweep slope, marginal body at 16-vs-8 bodies/iteration, min of
two interleaved passes): 1218ns vs the 1970ns bf16 baseline — on the byte
roofline (359KB/body at the ~300GB/s effective DMA bandwidth).  Deeper
unrolls read HIGHER (32 bodies/iter: 1367ns; sequencer instruction fetch),
and every second-input-DMA variant loses ~400-850ns regardless of ring
(SP/ACT/Pool), which is why everything rides in the single wx DMA.

Sharding: K=2048 row-sharded over 8 cores (256 classes each), x replicated.
"""

import os
import sys

import numpy as np

B, K, D = 128, 2048, 512
NCORES = 8
KLOC = K // NCORES          # 256 classes per core
DCH = D // 128              # 4 contraction chunks

# asinh(z) ~= z*(PG0 + PG1*z^2), minimax on |z| <= 0.68
PG0, PG1 = 0.99652869, -0.13320923
PD = PG0 / PG1

WXS = 16.0                  # W stored as e3m4*WXS; xs stored as bf16/WXS

# wx byte layout per partition ([128, WX_BYTES] uint8):
#   [0, 1024):     x chunks bf16: chunk c cols [c*256,(c+1)*256); partition d
#                  holds bf16 xs[b, c*128+d]/16 for b in 0..127
#   [1024, 2048):  W chunks e3m4: chunk c cols [1024+c*256, ...); partition d
#                  holds e3m4 16*W[k0+j, c*128+d] for j in 0..255
#   [2048, 2816):  rider strip: slot A = bf16[128] lhsT rows (256B), slot BC
#                  = bf16[256] rhs rows (512B).  Matmul operands must share a
#                  base partition in {0,32,64} (PE tile_position), so lhsT
#                  sits in slot A and rhs in slot BC of the same partitions:
#                    p0:     A=vws   BC=qscale            (z-rider)
#                    p64/65: A=1/1   BC=lhi/llo           (lam, hi/lo rows)
#                  where l = lam*PG1 hi/lo bf16 split.
X_OFF = 0
XCHB = 128                  # x chunk bytes (e3m4)
W_OFF = DCH * XCHB
R_OFF = W_OFF + DCH * 256
RB_OFF = R_OFF + 256
WX_BYTES = RB_OFF + 512
S_MM = 256.0                # psum holds S_MM * z (x16 per side)

_CACHE: dict = {}


def _import_concourse():
    try:
        import concourse.bass  # noqa: F401
    except ImportError:
        for path in ("/opt/trn_rl_repo", os.path.expanduser("~/.axon_site/_ro/trn_rl_repo")):
            if os.path.isdir(path) and path not in sys.path:
                sys.path.insert(0, path)
        import concourse.bass  # noqa: F401


def _build_nc(bench_loop=None):
    """Build the single-core Bass/Tile program (same program for all 8 cores).

    bench_loop=(n_iters, reps): wrap the body in a For_i hardware loop
    (timing harness).  bench_loop=(0, reps): straight-line unroll for the
    local TimelineSim (it cannot simulate For_i branches).
    """
    import concourse.tile as tile
    from concourse import bacc, mybir
    from concourse.alu_op_type import AluOpType

    f32 = mybir.dt.float32
    bf16 = mybir.dt.bfloat16
    f8e3 = mybir.dt.float8e3
    u8 = mybir.dt.uint8

    nc = bacc.Bacc("TRN2", target_bir_lowering=False, debug=False, num_devices=NCORES)
    wx_d = nc.declare_dram_parameter("wx", [128, WX_BYTES], u8, isOutput=False)
    n_outs = 1 if bench_loop is None else max(1, bench_loop[1])
    out_ds = [
        nc.declare_dram_parameter(
            "out" if r == 0 else f"out{r}", [128, KLOC], bf16, isOutput=True
        )
        for r in range(n_outs)
    ]

    with tile.TileContext(nc) as tc:
        with (
            tc.tile_pool(name="sbuf", bufs=6) as pool,
            tc.tile_pool(name="psum", bufs=4, space="PSUM") as pp,
        ):
            def load_act_table():
                nc.scalar.add_instruction(
                    mybir.InstLoadActFuncSet(
                        name=nc.get_next_instruction_name(),
                        ins=[],
                        outs=[],
                        act_func_set_id=6,
                    )
                )

            def emit(out_d, load_table=False):
                wx_sb = pool.tile([128, WX_BYTES], u8)
                nc.sync.dma_start(wx_sb[:], wx_d[:])
                if load_table:
                    load_act_table()

                def xh(c):
                    return wx_sb[:, X_OFF + c * XCHB : X_OFF + (c + 1) * XCHB].bitcast(f8e3)

                def wh(c):
                    return wx_sb[:, W_OFF + c * 256 : W_OFF + (c + 1) * 256].bitcast(f8e3)

                def ra(p, np_=1):
                    return wx_sb[p : p + np_, R_OFF : R_OFF + 256].bitcast(bf16)

                def rbc(p, np_=1):
                    return wx_sb[p : p + np_, RB_OFF : RB_OFF + 512].bitcast(bf16)

                # The z-rider starts the accumulation; it and the lam rider
                # are gated only on the wx DMA and run before the mains on
                # the in-order PE ring, so the mains' stop covers every PSUM
                # producer with one semaphore.
                mm = pp.tile([128, KLOC], f32)
                nc.tensor.matmul(
                    mm[:], ra(0), rbc(0), start=True, stop=False,
                )
                lam_ps = pp.tile([128, KLOC], f32)
                nc.tensor.matmul(
                    lam_ps[:], ra(64, 2), rbc(64, 2),
                    start=True, stop=True, skip_group_check=True,
                )
                # Main matmuls: mixed dtype (stationary x bf16, moving W e3m4)
                for c in range(DCH):
                    nc.tensor.matmul(
                        mm[:],
                        xh(c),
                        wh(c),
                        start=False,
                        stop=(c == DCH - 1),
                        skip_group_check=True,
                    )

                # ACT stages z and z^2 out of PSUM (a DVE op may read at most
                # one PSUM operand; zl's is lam_ps).  All f32: the fused STT
                # has no 16-bit fast mode, so f32 costs the same time.
                zc_sb = pool.tile([128, KLOC], f32)
                nc.scalar.copy(zc_sb[:], mm[:])
                u_sb = pool.tile([128, KLOC], f32)
                nc.scalar.square(u_sb[:], mm[:])
                zl_sb = pool.tile([128, KLOC], f32)
                nc.vector.tensor_mul(zl_sb[:], zc_sb[:], lam_ps[:])
                o_sb = pool.tile([128, KLOC], bf16)
                nc.vector.scalar_tensor_tensor(
                    o_sb[:], u_sb[:], float(PD * S_MM * S_MM), zl_sb[:],
                    AluOpType.add, AluOpType.mult,
                )
                nc.gpsimd.dma_start(out_d[:], o_sb[:])

            if bench_loop is None:
                emit(out_ds[0], load_table=True)
            else:
                n_iters, reps = bench_loop
                load_act_table()
                if n_iters == 0:
                    for r in range(reps):
                        emit(out_ds[r])
                else:
                    with tc.For_i(0, n_iters, 1):
                        for r in range(reps):
                            emit(out_ds[r])

    nc.compile()
    return nc


def _host_prep(inp, p, a):
    """Host-side constant folding / layout prep. Returns per-core input maps."""
    import ml_dtypes

    bf = ml_dtypes.bfloat16
    e3 = ml_dtypes.float8_e3m4
    inp64 = inp.astype(np.float64)
    p64 = p.astype(np.float64)
    a64 = a.astype(np.float64)

    vv = np.sum(inp64 * inp64, axis=1)            # [B]
    winv = 1.0 / (1.0 - vv)
    vws = (1.0 + vv) * winv

    uu = np.sum(p64 * p64, axis=1)                # [K]
    beta = 1.0 - uu
    ua = -np.sum(p64 * a64, axis=1)
    an = np.sqrt(np.sum(a64 * a64, axis=1))
    qscale = 2.0 * ua / (an * beta)
    lam = 2.0 * an / beta
    W = (-2.0 * qscale)[:, None] * p64 + (2.0 / an)[:, None] * a64   # [K, D]

    xs = inp64 * winv[:, None]                    # [B, D]

    # x packed e3m4*16: xh_p[d, c*128+b] = 16*xs[b, c*128+d]; psum = 256*z
    xq = (xs * 16.0).astype(e3)                   # [B, D]
    xh_p = np.ascontiguousarray(
        xq.T.reshape(DCH, 128, B).transpose(1, 0, 2).reshape(128, DCH * B)
    )                                             # [128, 512] bf16
    x_bytes = xh_p.view(np.uint8)                 # [128, 1024]

    Wq = (W * WXS).astype(e3)                     # [K, D] e3m4
    lamP = lam * PG1 / (256.0**3)   # folds the x256 psum unscale

    in_maps = []
    for i in range(NCORES):
        k0 = i * KLOC
        # W: wh_p[d, c, j] = Wq[k0+j, c*128+d]
        wh_p = Wq[k0 : k0 + KLOC].T.reshape(DCH, 128, KLOC).transpose(1, 0, 2)
        w_bytes = np.ascontiguousarray(wh_p.reshape(128, DCH * KLOC)).view(np.uint8)

        r_a = np.zeros((128, 128), bf)            # slot A: lhsT rows
        r_bc = np.zeros((128, 256), bf)           # slot BC: rhs rows
        r_a[0] = vws.astype(bf)
        r_a[64] = r_a[65] = np.ones(128, bf)
        r_bc[0] = (qscale[k0 : k0 + KLOC] * 256.0).astype(bf)
        lh = lamP[k0 : k0 + KLOC]
        hi = lh.astype(bf)
        r_bc[64] = hi
        r_bc[65] = (lh - hi.astype(np.float64)).astype(bf)

        wx = np.empty((128, WX_BYTES), np.uint8)
        wx[:, X_OFF:W_OFF] = x_bytes
        wx[:, W_OFF:R_OFF] = w_bytes
        wx[:, R_OFF:RB_OFF] = r_a.view(np.uint8)
        wx[:, RB_OFF:] = r_bc.view(np.uint8)
        in_maps.append({"wx": np.ascontiguousarray(wx)})
    return in_maps


def _run(in_maps, trace=False, **kw):
    from concourse.bass_utils import run_bass_kernel_spmd

    if "nc" not in _CACHE:
        _CACHE["nc"] = _build_nc()
    return run_bass_kernel_spmd(
        _CACHE["nc"], in_maps, list(range(NCORES)), trace=trace, **kw
    )


def kernel(inp, p, a):
    _import_concourse()
    inp = np.asarray(inp, np.float32)
    p = np.asarray(p, np.float32)
    a = np.asarray(a, np.float32)
    in_maps = _host_prep(inp, p, a)
    res = _run(in_maps)
    out = np.concatenate(
        [np.asarray(res.results[i]["out"]) for i in range(NCORES)], axis=1
    )
    return out.astype(np.float32)


# revision 5
# speedup vs baseline: 1.0419x; 1.0419x over previous
"""Hyperbolic MLR logits (Ganea et al.) on 8 Trainium2 NeuronCores.

Shapes (hardcoded): inp [128, 512] f32, p [2048, 512] f32, a [2048, 512] f32,
output [128, 2048] f32.

Math (same collapse as the original baseline): with c=1 the reference
    logit[b,k] = lam_p[k] * ||a_k|| * asinh(2<w,a_k>/(||a_k||(1-||w||^2))),
    w = (-p_k) (+)_mobius x_b
reduces via the gyrovector identity to
    logit[b,k] = (z^2 + PG0/PG1) * (z * lam[k]*PG1),
    z[b,k]     = vws[b]*qscale[k] + <W_k, xs_b>          (deg-3 asinh poly)
with all coefficient vectors host-precomputed (see _host_prep).

Design — driven by knockout probes on the real device (not the cost model):
  * The measured marginal body tracks total DMA BYTES and the NUMBER of
    input DMAs.  A second input DMA costs ~400-500ns at depth no matter
    which ring issues it, so everything rides in ONE bulk DMA per body.
  * Both matmul operands are fp8 e3m4 scaled x16 into its normal range
    (PSUM holds 256*z; the unscale is folded into host constants — all
    powers of two, so exact): wx bytes per core drop 384KB -> 288KB... per
    body total (wx 295KB + out 64KB) = 359KB vs the bf16 baseline's 451KB.
    End-to-end absmax-rel error: 1.715e-2 (HW bit-identical to the host
    simulation) vs the 2e-2 gate.
  * The rank-1 z-term and the lam broadcast ride as bf16 "rider rows"
    inside the wx DMA (a 768B column strip; only partitions 0/64/65 carry
    data).  Rider matmuls need lhsT/rhs at the SAME base partition (in
    {0,32,64}), hence the slot-A (lhsT) / slot-BC (rhs) layout.
  * The asinh chain is balanced ACT/DVE, each ~2 ops/body:
      ACT zc = copy(mm)  (256*z), ACT u = square(mm)  (65536*z^2),
      DVE zl = zc * lam_ps  (lam rides as lam*PG1/2^24),
      DVE o  = (u + (PG0/PG1)*65536) * zl  -> bf16.
    Each instruction needs at most one fresh semaphore (in-order rings
    cover the rest).
  * Two PSUM banks per body (mm + lam_ps), bufs=4 -> 4-deep rotation;
    SBUF pool 6-deep.  PE order: z-rider, lam riders, then the 4 mains
    (mixed dtype is allowed; the mains are e3m4 x e3m4).
  * Output via the Pool/SWDGE ring (the two HWDGE rings would serialize).

Measured (n-sweep slope, marginal body at 12-vs-8 bodies/iteration, min of
two interleaved passes): 1218ns vs the 1970ns bf16 baseline — on the byte
roofline (359KB/body at the ~300GB/s effective DMA bandwidth).  Deeper
unrolls read HIGHER (32 bodies/iter: 1367ns; sequencer instruction fetch),
and every second-input-DMA variant loses ~400-850ns regardless of ring
(SP/ACT/Pool), which is why everything rides in the single wx DMA.

Sharding: K=2048 row-sharded over 8 cores (256 classes each), x replicated.
"""

import os
import sys

import numpy as np

B, K, D = 128, 2048, 512
NCORES = 8
KLOC = K // NCORES          # 256 classes per core
DCH = D // 128              # 4 contraction chunks

# asinh(z) ~= z*(PG0 + PG1*z^2), minimax on |z| <= 0.68
PG0, PG1 = 0.99652869, -0.13320923
PD = PG0 / PG1

WXS = 16.0                  # W stored as e3m4*WXS; xs stored as bf16/WXS

# wx byte layout per partition ([128, WX_BYTES] uint8):
#   [0, 1024):     x chunks bf16: chunk c cols [c*256,(c+1)*256); partition d
#                  holds bf16 xs[b, c*128+d]/16 for b in 0..127
#   [1024, 2048):  W chunks e3m4: chunk c cols [1024+c*256, ...); partition d
#                  holds e3m4 16*W[k0+j, c*128+d] for j in 0..255
#   [2048, 2816):  rider strip: slot A = bf16[128] lhsT rows (256B), slot BC
#                  = bf16[256] rhs rows (512B).  Matmul operands must share a
#                  base partition in {0,32,64} (PE tile_position), so lhsT
#                  sits in slot A and rhs in slot BC of the same partitions:
#                    p0:     A=vws   BC=qscale            (z-rider)
#                    p64/65: A=1/1   BC=lhi/llo           (lam, hi/lo rows)
#                  where l = lam*PG1 hi/lo bf16 split.
X_OFF = 0
XCHB = 128                  # x chunk bytes (e3m4)
W_OFF = DCH * XCHB
R_OFF = W_OFF + DCH * 256
RB_OFF = R_OFF + 256
WX_BYTES = RB_OFF + 512
S_MM = 256.0                # psum holds S_MM * z (x16 per side)

_CACHE: dict = {}


def _import_concourse():
    try:
        import concourse.bass  # noqa: F401
    except ImportError:
        for path in ("/opt/trn_rl_repo", os.path.expanduser("~/.axon_site/_ro/trn_rl_repo")):
            if os.path.isdir(path) and path not in sys.path:
                sys.path.insert(0, path)
        import concourse.bass  # noqa: F401


def _build_nc(bench_loop=None):
    """Build the single-core Bass/Tile program (same program for all 8 cores).

    bench_loop=(n_iters, reps): wrap the body in a For_i hardware loop
    (timing harness).  bench_loop=(0, reps): straight-line unroll for the
    local TimelineSim (it cannot simulate For_i branches).
    """
    import concourse.tile as tile
    from concourse import bacc, mybir
    from concourse.alu_op_type import AluOpType

    f32 = mybir.dt.float32
    bf16 = mybir.dt.bfloat16
    f8e3 = mybir.dt.float8e3
    u8 = mybir.dt.uint8

    nc = bacc.Bacc("TRN2", target_bir_lowering=False, debug=False, num_devices=NCORES)
    wx_d = nc.declare_dram_parameter("wx", [128, WX_BYTES], u8, isOutput=False)
    n_outs = 1 if bench_loop is None else max(1, bench_loop[1])
    out_ds = [
        nc.declare_dram_parameter(
            "out" if r == 0 else f"out{r}", [128, KLOC], bf16, isOutput=True
        )
        for r in range(n_outs)
    ]

    with tile.TileContext(nc) as tc:
        with (
            tc.tile_pool(name="sbuf", bufs=6) as pool,
            tc.tile_pool(name="psum", bufs=4, space="PSUM") as pp,
        ):
            def load_act_table():
                nc.scalar.add_instruction(
                    mybir.InstLoadActFuncSet(
                        name=nc.get_next_instruction_name(),
                        ins=[],
                        outs=[],
                        act_func_set_id=6,
                    )
                )

            def emit(out_d, load_table=False):
                wx_sb = pool.tile([128, WX_BYTES], u8)
                nc.sync.dma_start(wx_sb[:], wx_d[:])
                if load_table:
                    load_act_table()

                def xh(c):
                    return wx_sb[:, X_OFF + c * XCHB : X_OFF + (c + 1) * XCHB].bitcast(f8e3)

                def wh(c):
                    return wx_sb[:, W_OFF + c * 256 : W_OFF + (c + 1) * 256].bitcast(f8e3)

                def ra(p, np_=1):
                    return wx_sb[p : p + np_, R_OFF : R_OFF + 256].bitcast(bf16)

                def rbc(p, np_=1):
                    return wx_sb[p : p + np_, RB_OFF : RB_OFF + 512].bitcast(bf16)

                # The z-rider starts the accumulation; it and the lam rider
                # are gated only on the wx DMA and run before the mains on
                # the in-order PE ring, so the mains' stop covers every PSUM
                # producer with one semaphore.
                mm = pp.tile([128, KLOC], f32)
                nc.tensor.matmul(
                    mm[:], ra(0), rbc(0), start=True, stop=False,
                )
                lam_ps = pp.tile([128, KLOC], f32)
                nc.tensor.matmul(
                    lam_ps[:], ra(64, 2), rbc(64, 2),
                    start=True, stop=True, skip_group_check=True,
                )
                # Main matmuls: mixed dtype (stationary x bf16, moving W e3m4)
                for c in range(DCH):
                    nc.tensor.matmul(
                        mm[:],
                        xh(c),
                        wh(c),
                        start=False,
                        stop=(c == DCH - 1),
                        skip_group_check=True,
                    )

                # ACT stages z and z^2 out of PSUM (a DVE op may read at most
                # one PSUM operand; zl's is lam_ps).  All f32: the fused STT
                # has no 16-bit fast mode, so f32 costs the same time.
                zc_sb = pool.tile([128, KLOC], f32)
                nc.scalar.copy(zc_sb[:], mm[:])
                u_sb = pool.tile([128, KLOC], f32)
                nc.scalar.square(u_sb[:], mm[:])
                zl_sb = pool.tile([128, KLOC], f32)
                nc.vector.tensor_mul(zl_sb[:], zc_sb[:], lam_ps[:])
                o_sb = pool.tile([128, KLOC], bf16)
                nc.vector.scalar_tensor_tensor(
                    o_sb[:], u_sb[:], float(PD * S_MM * S_MM), zl_sb[:],
                    AluOpType.add, AluOpType.mult,
                )
                nc.gpsimd.dma_start(out_d[:], o_sb[:])

            if bench_loop is None:
                emit(out_ds[0], load_table=True)
            else:
                n_iters, reps = bench_loop
                load_act_table()
                if n_iters == 0:
                    for r in range(reps):
                        emit(out_ds[r])
                else:
                    with tc.For_i(0, n_iters, 1):
                        for r in range(reps):
                            emit(out_ds[r])

    nc.compile()
    return nc


def _host_prep(inp, p, a):
    """Host-side constant folding / layout prep. Returns per-core input maps."""
    import ml_dtypes

    bf = ml_dtypes.bfloat16
    e3 = ml_dtypes.float8_e3m4
    inp64 = inp.astype(np.float64)
    p64 = p.astype(np.float64)
    a64 = a.astype(np.float64)

    vv = np.sum(inp64 * inp64, axis=1)            # [B]
    winv = 1.0 / (1.0 - vv)
    vws = (1.0 + vv) * winv

    uu = np.sum(p64 * p64, axis=1)                # [K]
    beta = 1.0 - uu
    ua = -np.sum(p64 * a64, axis=1)
    an = np.sqrt(np.sum(a64 * a64, axis=1))
    qscale = 2.0 * ua / (an * beta)
    lam = 2.0 * an / beta
    W = (-2.0 * qscale)[:, None] * p64 + (2.0 / an)[:, None] * a64   # [K, D]

    xs = inp64 * winv[:, None]                    # [B, D]

    # x packed e3m4*16: xh_p[d, c*128+b] = 16*xs[b, c*128+d]; psum = 256*z
    xq = (xs * 16.0).astype(e3)                   # [B, D]
    xh_p = np.ascontiguousarray(
        xq.T.reshape(DCH, 128, B).transpose(1, 0, 2).reshape(128, DCH * B)
    )                                             # [128, 512] bf16
    x_bytes = xh_p.view(np.uint8)                 # [128, 1024]

    Wq = (W * WXS).astype(e3)                     # [K, D] e3m4
    lamP = lam * PG1 / (256.0**3)   # folds the x256 psum unscale

    in_maps = []
    for i in range(NCORES):
        k0 = i * KLOC
        # W: wh_p[d, c, j] = Wq[k0+j, c*128+d]
        wh_p = Wq[k0 : k0 + KLOC].T.reshape(DCH, 128, KLOC).transpose(1, 0, 2)
        w_bytes = np.ascontiguousarray(wh_p.reshape(128, DCH * KLOC)).view(np.uint8)

        r_a = np.zeros((128, 128), bf)            # slot A: lhsT rows
        r_bc = np.zeros((128, 256), bf)           # slot BC: rhs rows
        r_a[0] = vws.astype(bf)
        r_a[64] = r_a[65] = np.ones(128, bf)
        r_bc[0] = (qscale[k0 : k0 + KLOC] * 256.0).astype(bf)
        lh = lamP[k0 : k0 + KLOC]
        hi = lh.astype(bf)
        r_bc[64] = hi
        r_bc[65] = (lh - hi.astype(np.float64)).astype(bf)

        wx = np.empty((128, WX_BYTES), np.uint8)
        wx[:, X_OFF:W_OFF] = x_bytes
        wx[:, W_OFF:R_OFF] = w_bytes
        wx[:, R_OFF:RB_OFF] = r_a.view(np.uint8)
        wx[:, RB_OFF:] = r_bc.view(np.uint8)
        in_maps.append({"wx": np.ascontiguousarray(wx)})
    return in_maps


def _run(in_maps, trace=False, **kw):
    from concourse.bass_utils import run_bass_kernel_spmd

    if "nc" not in _CACHE:
        _CACHE["nc"] = _build_nc()
    return run_bass_kernel_spmd(
        _CACHE["nc"], in_maps, list(range(NCORES)), trace=trace, **kw
    )


def kernel(inp, p, a):
    _import_concourse()
    inp = np.asarray(inp, np.float32)
    p = np.asarray(p, np.float32)
    a = np.asarray(a, np.float32)
    in_maps = _host_prep(inp, p, a)
    res = _run(in_maps)
    out = np.concatenate(
        [np.asarray(res.results[i]["out"]) for i in range(NCORES)], axis=1
    )
    return out.astype(np.float32)
